# revision 15
# baseline (speedup 1.0000x reference)
"""Causal multi-head attention block (b=8, s=1024, d_model=768, 12 heads x 64)
on 8 TRN2 NeuronCores - batch-parallel: core i computes batch element i.

Self-contained: includes the NTFF-profile-hook shim and the BIR wait-split
workaround for this walrus build (max 1 semaphore wait per instruction).

Per-core plan (bf16 matmuls, fp32 PSUM accumulation):
  A. x arrives bf16 pre-transposed; batched DMA (1 per xT tile, 1 per weight
     matrix, 1 for all small tensors) spread across engine queues.
  B. QT/KT [hd-blk][128,1024] = W.T @ xT (head-pair packed); V in natural
     [s,hd] layout padded to 65 cols/head with a ones column (rowsum trick)
  C. per q-half(512) / head-pair: scoresT[k,q] = KT.T @ QT on PE (two heads
     concurrent via row-group tiling), exp on ACT (1/8 scale folded in),
     causal handled by narrowing all ops to the live q-range per k-tile plus
     [128,128] triangular gpsimd affine_select masks on diagonal tiles only;
     PV accumulates [65,512]x2 banks (row 64 = softmax denominator).
  D. denominators batch into a [12, 512] tile per q-half; ONE DVE reciprocal,
     broadcast via indsel matmul (one per head-pair), one [128,512] multiply
     per pair; out-proj + b_O; DMA out on sync queue.
"""

import os
import sys
import types

import numpy as np

# ---------------------------------------------------------------------------
# environment shims


def _install_ntff_hook():
    try:
        import antenv
        from trn_agent_boot.trn_boot import _ntff_profile_via_ctypes
    except Exception:
        return
    if "antenv.axon_hooks" in sys.modules:
        return
    hook = _ntff_profile_via_ctypes("/opt/axon/libaxon_pjrt.so")
    m = types.ModuleType("antenv.axon_hooks")
    m.set_axon_ntff_profile_hook = lambda h: None
    m.get_axon_ntff_profile_hook = lambda: hook
    sys.modules["antenv.axon_hooks"] = m
    antenv.axon_hooks = m


def _install_waitsplit(max_waits=1):
    """walrus on this build rejects >1 sem wait per instruction; split extras
    onto preceding NoOps (same engine, program order preserved)."""
    import json

    import concourse.bass as bass

    if getattr(bass.Bass, "_waitsplit_installed", False):
        return
    counter = [0]

    def _split(inst):
        si = inst.get("sync_info")
        if not si:
            return [inst]
        waits = si.get("on_wait") or []
        if len(waits) <= max_waits:
            return [inst]
        out = []
        head, rest = waits[:-max_waits], waits[-max_waits:]
        for i in range(0, len(head), max_waits):
            counter[0] += 1
            out.append(
                {
                    "debug": inst.get("debug", 0),
                    "engine": inst["engine"],
                    "ins": [],
                    "name": f"I-waitsplit-{counter[0]}",
                    "opcode": "NoOp",
                    "outs": [],
                    "text_hint": "waitsplit",
                    "sync_info": {
                        "on_update": [],
                        "on_wait": head[i : i + max_waits],
                    },
                }
            )
        si["on_wait"] = rest
        out.append(inst)
        return out

    orig = bass.Bass.to_json_bytes

    def to_json_bytes(self):
        d = json.loads(orig(self))
        changed = False
        for f in d.get("functions", []):
            for bb in f.get("blocks", []):
                new = []
                for inst in bb.get("instructions", []):
                    parts = _split(inst)
                    changed = changed or len(parts) > 1
                    new.extend(parts)
                bb["instructions"] = new
        return json.dumps(d).encode() if changed else orig(self)

    bass.Bass.to_json_bytes = to_json_bytes
    bass.Bass._waitsplit_installed = True


_install_ntff_hook()
_install_waitsplit()

import ml_dtypes  # noqa: E402
import concourse.bass as bass  # noqa: E402
import concourse.mybir as mybir  # noqa: E402
import concourse.tile as tile  # noqa: E402
from concourse.bass_utils import run_bass_kernel_spmd  # noqa: E402

# ---------------------------------------------------------------------------
# problem constants (hardcoded per harness contract)

B, S, D, H, DH = 8, 1024, 768, 12, 64
P = 128
MT = D // P            # 6 tiles over d_model / hd
QH = 512               # q-half width
NKT = S // P           # 8 k-tiles over seq
SCALE = float(1.0 / np.sqrt(DH))
N_CORES = 8

F32 = mybir.dt.float32
F32R = mybir.dt.float32r
BF16 = mybir.dt.bfloat16
MMDT = BF16


def build_nc() -> bass.Bass:
    nc = bass.Bass()
    xT = nc.declare_dram_parameter("xT", [D, S], MMDT, isOutput=False)
    wq = nc.declare_dram_parameter("wq", [D, D], MMDT, isOutput=False)
    wk = nc.declare_dram_parameter("wk", [D, D], MMDT, isOutput=False)
    wv = nc.declare_dram_parameter("wv", [D, D], MMDT, isOutput=False)
    wo = nc.declare_dram_parameter("wo", [D, D], MMDT, isOutput=False)
    # smalls rows: 0 bq, 1 bk, 2 bv, 3 bo
    smalls = nc.declare_dram_parameter("smalls", [4, D], F32, isOutput=False)
    y = nc.declare_dram_parameter("y", [S, D], F32, isOutput=True)

    Exp = mybir.ActivationFunctionType.Exp
    mult = mybir.AluOpType.mult
    add = mybir.AluOpType.add
    is_ge = mybir.AluOpType.is_ge

    from contextlib import ExitStack

    with ExitStack() as _ctx:
        tc = _ctx.enter_context(tile.TileContext(nc))
        constp = _ctx.enter_context(tc.tile_pool(name="const", bufs=1))
        xtp = _ctx.enter_context(tc.tile_pool(name="xT", bufs=1))
        qtp = _ctx.enter_context(tc.tile_pool(name="qt", bufs=1))
        ktp = _ctx.enter_context(tc.tile_pool(name="kt", bufs=1))
        vpp = _ctx.enter_context(tc.tile_pool(name="vp", bufs=1))
        wtsp = _ctx.enter_context(tc.tile_pool(name="wts", bufs=4))
        expp = _ctx.enter_context(tc.tile_pool(name="expst", bufs=8))
        wsp = _ctx.enter_context(tc.tile_pool(name="wstack", bufs=12))
        outp = _ctx.enter_context(tc.tile_pool(name="outsb", bufs=2))
        smallp = _ctx.enter_context(tc.tile_pool(name="small", bufs=2))
        psflow = _ctx.enter_context(
            tc.tile_pool(name="ps_flow", bufs=2, space="PSUM")
        )
        psacc = _ctx.enter_context(
            tc.tile_pool(name="ps_acc", bufs=1, space="PSUM")
        )
        scpp = _ctx.enter_context(
            tc.tile_pool(name="ps_scp", bufs=2, space="PSUM")
        )

        # ---- batched input DMAs, spread across engine queues ---------------
        xts = [
            xtp.tile([P, S], MMDT, tag=f"xT{mt}", name=f"xT{mt}")
            for mt in range(MT)
        ]
        for mt in range(3):
            nc.sync.dma_start(xts[mt][:], xT[mt * P : (mt + 1) * P, :])
        for mt in range(3, MT):
            nc.scalar.dma_start(xts[mt][:], xT[mt * P : (mt + 1) * P, :])

        wq_all = wtsp.tile([P, MT * D], MMDT, tag="w", name="wq_all")
        wk_all = wtsp.tile([P, MT * D], MMDT, tag="w", name="wk_all")
        wv_all = wtsp.tile([P, MT * D], MMDT, tag="w", name="wv_all")
        wo_all = wtsp.tile([P, MT * D], MMDT, tag="w", name="wo_all")
        for eng, wall, dram in (
            (nc.sync, wq_all, wq),
            (nc.sync, wk_all, wk),
            (nc.gpsimd, wv_all, wv),
            (nc.gpsimd, wo_all, wo),
        ):
            eng.dma_start(
                wall.rearrange("p (t d) -> p t d", d=D),
                dram.rearrange("(t p) d -> p t d", p=P),
            )

        # each small row into its own partition-0 staging tile (partition
        # offsets other than 0/32/64/96 are illegal for DVE reads)
        bstages = [
            constp.tile([1, D], F32, tag=f"bstage{i}", name=f"bstage{i}")
            for i in range(4)
        ]
        for i in range(4):
            nc.gpsimd.dma_start(bstages[i][:], smalls[i : i + 1, :])

        def wt(wall, mt, c0, c1):
            return wall[:, mt * D + c0 : mt * D + c1]

        # ---- constants -----------------------------------------------------
        ones_stage = constp.tile([1, P], F32, tag="onesstage")
        nc.vector.memset(ones_stage[:], 1.0)
        ones_row = constp.tile([1, P], F32R, tag="onesrow")
        nc.vector.tensor_copy(ones_row[:], ones_stage[:])
        ones_col = constp.tile([P, H], F32, tag="onescol")
        nc.vector.memset(ones_col[:], 1.0)

        bq_row = constp.tile([1, D], F32R, tag="bqrow")
        bk_row = constp.tile([1, D], F32R, tag="bkrow")
        bv_row = constp.tile([1, D], F32R, tag="bvrow")
        bo_row = constp.tile([1, D], F32R, tag="borow")
        for row, stage in zip((bq_row, bk_row, bv_row, bo_row), bstages):
            nc.vector.tensor_copy(row[:], stage[:])

        # bq/bk as per-partition bias columns: transpose each 128-chunk of the
        # bias row via a tiny N=1 matmul (lhsT = row slice, rhs = ones[1,1])
        bq_t = constp.tile([P, MT], F32, tag="bq")  # col hdb = bias block
        bk_t = constp.tile([P, MT], F32, tag="bk")
        bcol_ps = psflow.tile([P, 512], F32, tag="ps", name="bcol_ps")
        for hdb in range(MT):
            nc.tensor.matmul(
                bcol_ps[:, 2 * hdb : 2 * hdb + 2],
                bq_row[:, hdb * P : (hdb + 1) * P],
                ones_row[:, 0:2],
                start=True, stop=True,
            )
            nc.tensor.matmul(
                bcol_ps[:, 2 * MT + 2 * hdb : 2 * MT + 2 * hdb + 2],
                bk_row[:, hdb * P : (hdb + 1) * P],
                ones_row[:, 0:2],
                start=True, stop=True,
            )
        for dst, c0 in ((bq_t, 0), (bk_t, 2 * MT)):
            src = bass.AP(
                bcol_ps.tensor, bcol_ps.offset + c0,
                [bcol_ps.ap[0], [2, MT]],
            )
            nc.vector.tensor_copy(dst[:], src)

        # broadcast bv/bo rows to all partitions via K=1 outer-product matmul
        bv_b = constp.tile([P, D], F32, tag="bvb")
        bo_b = constp.tile([P, D], F32, tag="bob")
        for row, bcast in ((bv_row, bv_b), (bo_row, bo_b)):
            for c0, c1 in ((0, 512), (512, 768)):
                bps = psflow.tile([P, 512], F32, tag="ps", name="bps")
                nc.tensor.matmul(
                    bps[:, : c1 - c0],
                    ones_row[:],
                    row[:, c0:c1],
                    start=True,
                    stop=True,
                )
                nc.vector.tensor_copy(bcast[:, c0:c1], bps[:, : c1 - c0])

        # ---- projections ---------------------------------------------------
        qts = [qtp.tile([P, S], MMDT, tag=f"qt{i}", name=f"qt{i}") for i in range(MT)]
        kts = [ktp.tile([P, S], MMDT, tag=f"kt{i}", name=f"kt{i}") for i in range(MT)]
        vps = [
            vpp.tile([P, H * 65], MMDT, tag=f"vp{st}", name=f"vp{st}")
            for st in range(NKT)
        ]

        def proj_qk_piece(wall, b_t, dst, sc, hdb):
            s0 = sc * 512
            ps0 = psflow.tile([P, 512], F32, tag="ps", name="pj0")
            for mt in range(MT):
                nc.tensor.matmul(
                    ps0[:], wt(wall, mt, hdb * P, (hdb + 1) * P),
                    xts[mt][:, s0 : s0 + 512],
                    start=(mt == 0), stop=(mt == MT - 1),
                )
            bsl = b_t[:, hdb : hdb + 1]
            bb = bass.AP(bsl.tensor, bsl.offset, [bsl.ap[0], [0, 512]])
            nc.vector.tensor_tensor(
                dst[hdb][:, s0 : s0 + 512], ps0[:], bb, op=add
            )

        def proj_qk_chunk(wall, b_t, dst, sc):
            for hdb in range(MT):
                proj_qk_piece(wall, b_t, dst, sc, hdb)

        def proj_v(st):
            vv = vps[st].rearrange("p (h c) -> p h c", c=65)
            nc.vector.tensor_copy(
                vv[:, :, 64:65],
                ones_col.rearrange("p (h c) -> p h c", c=1),
            )
            ps0 = psflow.tile([P, 512], F32, tag="ps", name="pv0")
            ps1 = psflow.tile([P, 512], F32, tag="ps", name="pv1")
            for mt in range(MT):
                lx = xts[mt][:, st * P : (st + 1) * P]
                nc.tensor.matmul(
                    ps0[:], lx, wt(wv_all, mt, 0, 512),
                    start=(mt == 0), stop=(mt == MT - 1),
                )
                nc.tensor.matmul(
                    ps1[:, 0:256], lx, wt(wv_all, mt, 512, 768),
                    start=(mt == 0), stop=(mt == MT - 1),
                )
            bsrc = bv_b.rearrange("p (h c) -> p h c", c=DH)
            nc.vector.tensor_tensor(
                vv[:, 0:8, 0:DH],
                ps0.rearrange("p (h c) -> p h c", c=DH),
                bsrc[:, 0:8, :],
                op=add,
            )
            nc.vector.tensor_tensor(
                vv[:, 8:12, 0:DH],
                ps1[:, 0:256].rearrange("p (h c) -> p h c", c=DH),
                bsrc[:, 8:12, :],
                op=add,
            )

        # ---- attention core -------------------------------------------------
        # per (pp, hp): loop k-tiles; causal handled by narrowing every op to
        # the live q-range [lo:QH] (lo = 128*kt - 512*pp clamped at 0) plus a
        # [128,128] triangular mask on diagonal tiles only.
        def attn_core(pp, hp, wstack):
            q0 = pp * QH
            nkt1 = 4 * pp + 4
            pvs = psacc.tile([65, 2 * QH], F32, tag="pv", name=f"pv{pp}_{hp}")
            for kt in range(nkt1):
                off = P * kt - QH * pp
                lo = max(0, off)
                w = QH - lo
                scp = scpp.tile([P, 2 * QH], F32, tag="scp", name="scp")
                # the pair's two matmuls sit on disjoint PE row groups and
                # disjoint PSUM banks of one 2-bank tile
                for sub in range(2):
                    r0 = sub * 64
                    nc.tensor.matmul(
                        scp[:, sub * QH + lo : (sub + 1) * QH],
                        kts[hp][r0 : r0 + 64, kt * P : (kt + 1) * P],
                        qts[hp][r0 : r0 + 64, q0 + lo : q0 + QH],
                        start=True,
                        stop=True,
                        tile_position=(r0, 0),
                    )
                est = expp.tile([P, 2 * QH], MMDT, tag="est", name="est")
                if lo == 0:
                    nc.scalar.activation(est[:], scp[:], Exp, scale=SCALE)
                else:
                    sin = bass.AP(
                        scp.tensor, scp.offset + lo,
                        [scp.ap[0], [QH, 2], [1, w]],
                    )
                    sout = bass.AP(
                        est.tensor, est.offset + lo,
                        [est.ap[0], [QH, 2], [1, w]],
                    )
                    nc.scalar.activation(sout, sin, Exp, scale=SCALE)
                if off >= 0:
                    # diagonal tile: triangular mask on cols [off:off+P]
                    for sub in range(2):
                        b0 = sub * QH + off
                        nc.gpsimd.affine_select(
                            est[:, b0 : b0 + P], est[:, b0 : b0 + P],
                            pattern=[[1, P]],
                            compare_op=is_ge, fill=0.0,
                            base=0,
                            channel_multiplier=-1,
                        )
                for sub in range(2):
                    h = 2 * hp + sub
                    nc.tensor.matmul(
                        pvs[:, sub * QH + lo : (sub + 1) * QH],
                        vps[kt][:, h * 65 : (h + 1) * 65],
                        est[:, sub * QH + lo : (sub + 1) * QH],
                        start=(kt == 0),
                        stop=(kt == nkt1 - 1),
                        skip_group_check=True,
                    )
            # immediate stash frees the PV banks: unnormalized rows into
            # wstack (bf16); reciprocal of the denominator row straight off
            # PSUM partition 64 (32-aligned base) into an f32r row
            for sub in range(2):
                nc.vector.tensor_copy(
                    wstack[hp][sub * 64 : (sub + 1) * 64, :],
                    pvs[0:64, sub * QH : (sub + 1) * QH],
                )
            srec = smallp.tile([1, 2 * QH], F32R, tag="srec", bufs=8,
                               name=f"srec{pp}_{hp}")
            with nc.allow_low_precision(reason="softmax recip to f32r"):
                nc.vector.reciprocal(srec[:], pvs[64:65, :])
            return srec

        def norm_apply(hp, srec, wstack):
            # broadcast each head's reciprocal row to 64 partitions via a
            # K=1 matmul (PSUM dst must start at partition 0), then scale
            for sub in range(2):
                rb = psflow.tile([P, 512], F32, tag="ps", name="rb")
                nc.tensor.matmul(
                    rb[0:64, :],
                    ones_row[:, 0:64],
                    srec[:, sub * QH : (sub + 1) * QH],
                    start=True, stop=True,
                )
                nc.vector.tensor_tensor(
                    wstack[hp][sub * 64 : (sub + 1) * 64, :],
                    wstack[hp][sub * 64 : (sub + 1) * 64, :],
                    rb[0:64, :], op=mult,
                )

        def outproj_sub(pp, wstack, sub):
            q0 = pp * QH
            opsa = psflow.tile([P, 512], F32, tag="ps", name="opa_t")
            opsb = psflow.tile([P, 512], F32, tag="ps", name="opb_t")
            for hdt in range(MT):
                lw = wstack[hdt][:, sub * P : (sub + 1) * P]
                nc.tensor.matmul(
                    opsa[:], lw, wt(wo_all, hdt, 0, 512),
                    start=(hdt == 0), stop=(hdt == MT - 1),
                )
                nc.tensor.matmul(
                    opsb[:, 0:256], lw, wt(wo_all, hdt, 512, 768),
                    start=(hdt == 0), stop=(hdt == MT - 1),
                )
            osb = outp.tile([P, D], F32, tag="osb")
            nc.vector.tensor_tensor(
                osb[:, 0:512], opsa[:], bo_b[:, 0:512], op=add
            )
            nc.vector.tensor_tensor(
                osb[:, 512:768], opsb[:, 0:256], bo_b[:, 512:768], op=add
            )
            nc.sync.dma_start(
                y[q0 + sub * P : q0 + (sub + 1) * P, :], osb[:]
            )

        # ---- emission order -------------------------------------------------
        proj_qk_chunk(wq_all, bq_t, qts, 0)
        proj_qk_chunk(wk_all, bk_t, kts, 0)
        for st in range(4):
            proj_v(st)

        wstack0 = [
            wsp.tile([P, QH], MMDT, tag="ws", name=f"ws0_{i}")
            for i in range(MT)
        ]
        wstack1 = [
            wsp.tile([P, QH], MMDT, tag="ws", name=f"ws1_{i}")
            for i in range(MT)
        ]
        # q-half 0 attention; interleave half-1 projections between head pairs
        srecs0 = []
        for hp in range(MT):
            srecs0.append(attn_core(0, hp, wstack0))
            proj_qk_piece(wq_all, bq_t, qts, 1, hp)
            proj_qk_piece(wk_all, bk_t, kts, 1, hp)
            if hp < 4:
                proj_v(4 + hp)

        # q-half 1 attention; interleave half-0 norm + out-projection
        srecs1 = []
        for hp in range(MT):
            srecs1.append(attn_core(1, hp, wstack1))
            if hp == 0:
                for hp0 in range(3):
                    norm_apply(hp0, srecs0[hp0], wstack0)
            elif hp == 1:
                for hp0 in range(3, MT):
                    norm_apply(hp0, srecs0[hp0], wstack0)
            else:
                outproj_sub(0, wstack0, hp - 2)
        for hp in range(MT):
            norm_apply(hp, srecs1[hp], wstack1)
        for sub in range(4):
            outproj_sub(1, wstack1, sub)
    return nc


_NC_CACHE = None
LAST_EXEC_NS = None


def _get_nc():
    global _NC_CACHE
    if _NC_CACHE is None:
        _NC_CACHE = build_nc()
    return _NC_CACHE


def kernel(
    normalized_resid_pre, W_Q, W_K, W_V, W_O, b_Q, b_K, b_V, b_O
) -> np.ndarray:
    global LAST_EXEC_NS
    bf = ml_dtypes.bfloat16
    x = np.asarray(normalized_resid_pre, np.float32)
    xT = np.ascontiguousarray(x.transpose(0, 2, 1)).astype(bf)  # [b, D, S]
    wq = np.asarray(W_Q, np.float32).transpose(1, 0, 2).reshape(D, D).astype(bf)
    wk = np.asarray(W_K, np.float32).transpose(1, 0, 2).reshape(D, D).astype(bf)
    wv = np.asarray(W_V, np.float32).transpose(1, 0, 2).reshape(D, D).astype(bf)
    wo = np.asarray(W_O, np.float32).reshape(D, D).astype(bf)
    smalls = np.zeros((4, D), np.float32)
    smalls[0] = np.asarray(b_Q, np.float32).reshape(D)
    smalls[1] = np.asarray(b_K, np.float32).reshape(D)
    smalls[2] = np.asarray(b_V, np.float32).reshape(D)
    smalls[3] = np.asarray(b_O, np.float32).reshape(D)

    nc = _get_nc()
    in_maps = [
        {
            "xT": xT[i],
            "wq": wq, "wk": wk, "wv": wv, "wo": wo,
            "smalls": smalls,
        }
        for i in range(N_CORES)
    ]
    trace = os.environ.get("KERNEL_TRACE", "0") == "1"
    res = run_bass_kernel_spmd(
        nc, in_maps, list(range(N_CORES)), trace=trace
    )
    LAST_EXEC_NS = res.exec_time_ns
    out = np.stack(
        [res.results[i]["y"].astype(np.float32) for i in range(N_CORES)], axis=0
    )
    return out


# revision 17
# speedup vs baseline: 1.3647x; 1.3647x over previous
"""Causal multi-head attention block (b=8, s=1024, d_model=768, 12 heads x 64)
on 8 TRN2 NeuronCores - batch-parallel: core i computes batch element i.

Self-contained: includes the NTFF-profile-hook shim and the BIR wait-split
workaround for this walrus build (max 1 semaphore wait per instruction).

Per-core plan (bf16 matmuls, fp32 PSUM accumulation):
  A. x arrives bf16 pre-transposed; batched DMA (1 per xT tile, 1 per weight
     matrix, 1 for all small tensors) spread across engine queues.
  B. QT/KT [hd-blk][128,1024] = W.T @ xT (head-pair packed); V in natural
     [s,hd] layout padded to 65 cols/head with a ones column (rowsum trick)
  C. per q-half(512) / head-pair: scoresT[k,q] = KT.T @ QT on PE (two heads
     concurrent via row-group tiling), exp on ACT (1/8 scale folded in),
     causal handled by narrowing all ops to the live q-range per k-tile plus
     [128,128] triangular gpsimd affine_select masks on diagonal tiles only;
     PV accumulates [65,512]x2 banks (row 64 = softmax denominator).
  D. denominators batch into a [12, 512] tile per q-half; ONE DVE reciprocal,
     broadcast via indsel matmul (one per head-pair), one [128,512] multiply
     per pair; out-proj + b_O; DMA out on sync queue.
"""

import os
import sys
import types

import numpy as np

# ---------------------------------------------------------------------------
# environment shims


def _install_ntff_hook():
    try:
        import antenv
        from trn_agent_boot.trn_boot import _ntff_profile_via_ctypes
    except Exception:
        return
    if "antenv.axon_hooks" in sys.modules:
        return
    hook = _ntff_profile_via_ctypes("/opt/axon/libaxon_pjrt.so")
    m = types.ModuleType("antenv.axon_hooks")
    m.set_axon_ntff_profile_hook = lambda h: None
    m.get_axon_ntff_profile_hook = lambda: hook
    sys.modules["antenv.axon_hooks"] = m
    antenv.axon_hooks = m


def _install_waitsplit(max_waits=1):
    """walrus on this build rejects >1 sem wait per instruction; split extras
    onto preceding NoOps (same engine, program order preserved)."""
    import json

    import concourse.bass as bass

    if getattr(bass.Bass, "_waitsplit_installed", False):
        return
    counter = [0]

    def _split(inst):
        si = inst.get("sync_info")
        if not si:
            return [inst]
        waits = si.get("on_wait") or []
        if len(waits) <= max_waits:
            return [inst]
        out = []
        head, rest = waits[:-max_waits], waits[-max_waits:]
        for i in range(0, len(head), max_waits):
            counter[0] += 1
            out.append(
                {
                    "debug": inst.get("debug", 0),
                    "engine": inst["engine"],
                    "ins": [],
                    "name": f"I-waitsplit-{counter[0]}",
                    "opcode": "NoOp",
                    "outs": [],
                    "text_hint": "waitsplit",
                    "sync_info": {
                        "on_update": [],
                        "on_wait": head[i : i + max_waits],
                    },
                }
            )
        si["on_wait"] = rest
        out.append(inst)
        return out

    orig = bass.Bass.to_json_bytes

    def to_json_bytes(self):
        d = json.loads(orig(self))
        changed = False
        for f in d.get("functions", []):
            for bb in f.get("blocks", []):
                new = []
                for inst in bb.get("instructions", []):
                    parts = _split(inst)
                    changed = changed or len(parts) > 1
                    new.extend(parts)
                bb["instructions"] = new
        return json.dumps(d).encode() if changed else orig(self)

    bass.Bass.to_json_bytes = to_json_bytes
    bass.Bass._waitsplit_installed = True


_install_ntff_hook()
_install_waitsplit()

import ml_dtypes  # noqa: E402
import concourse.bass as bass  # noqa: E402
import concourse.mybir as mybir  # noqa: E402
import concourse.tile as tile  # noqa: E402
from concourse.bass_utils import run_bass_kernel_spmd  # noqa: E402

# ---------------------------------------------------------------------------
# problem constants (hardcoded per harness contract)

B, S, D, H, DH = 8, 1024, 768, 12, 64
P = 128
MT = D // P            # 6 tiles over d_model / hd
QH = 512               # q-half width
NKT = S // P           # 8 k-tiles over seq
SCALE = float(1.0 / np.sqrt(DH))
N_CORES = 8

F32 = mybir.dt.float32
F32R = mybir.dt.float32r
BF16 = mybir.dt.bfloat16
MMDT = BF16


def build_nc() -> bass.Bass:
    nc = bass.Bass()
    xT = nc.declare_dram_parameter("xT", [D, S], MMDT, isOutput=False)
    wq = nc.declare_dram_parameter("wq", [D, D], MMDT, isOutput=False)
    wk = nc.declare_dram_parameter("wk", [D, D], MMDT, isOutput=False)
    wv = nc.declare_dram_parameter("wv", [D, D], MMDT, isOutput=False)
    wo = nc.declare_dram_parameter("wo", [D, D], MMDT, isOutput=False)
    # smalls rows: 0 bq, 1 bk, 2 bv, 3 bo
    smalls = nc.declare_dram_parameter("smalls", [4, D], F32, isOutput=False)
    y = nc.declare_dram_parameter("y", [S, D], F32, isOutput=True)

    Exp = mybir.ActivationFunctionType.Exp
    mult = mybir.AluOpType.mult
    add = mybir.AluOpType.add
    is_ge = mybir.AluOpType.is_ge

    from contextlib import ExitStack

    with ExitStack() as _ctx:
        tc = _ctx.enter_context(tile.TileContext(nc))
        constp = _ctx.enter_context(tc.tile_pool(name="const", bufs=1))
        xtp = _ctx.enter_context(tc.tile_pool(name="xT", bufs=1))
        qtp = _ctx.enter_context(tc.tile_pool(name="qt", bufs=1))
        ktp = _ctx.enter_context(tc.tile_pool(name="kt", bufs=1))
        vpp = _ctx.enter_context(tc.tile_pool(name="vp", bufs=1))
        wtsp = _ctx.enter_context(tc.tile_pool(name="wts", bufs=4))
        expp = _ctx.enter_context(tc.tile_pool(name="expst", bufs=8))
        wsp = _ctx.enter_context(tc.tile_pool(name="wstack", bufs=12))
        outp = _ctx.enter_context(tc.tile_pool(name="outsb", bufs=2))
        smallp = _ctx.enter_context(tc.tile_pool(name="small", bufs=2))
        psflow = _ctx.enter_context(
            tc.tile_pool(name="ps_flow", bufs=2, space="PSUM")
        )
        psacc = _ctx.enter_context(
            tc.tile_pool(name="ps_acc", bufs=1, space="PSUM")
        )
        scpp = _ctx.enter_context(
            tc.tile_pool(name="ps_scp", bufs=2, space="PSUM")
        )

        # ---- batched input DMAs, spread across engine queues ---------------
        xts = [
            xtp.tile([P, S], MMDT, tag=f"xT{mt}", name=f"xT{mt}")
            for mt in range(MT)
        ]
        for mt in range(3):
            nc.sync.dma_start(xts[mt][:], xT[mt * P : (mt + 1) * P, :])
        for mt in range(3, MT):
            nc.scalar.dma_start(xts[mt][:], xT[mt * P : (mt + 1) * P, :])

        wq_all = wtsp.tile([P, MT * D], MMDT, tag="w", name="wq_all")
        wk_all = wtsp.tile([P, MT * D], MMDT, tag="w", name="wk_all")
        wv_all = wtsp.tile([P, MT * D], MMDT, tag="w", name="wv_all")
        wo_all = wtsp.tile([P, MT * D], MMDT, tag="w", name="wo_all")
        # each small row into its own partition-0 staging tile (partition
        # offsets other than 0/32/64/96 are illegal for DVE reads)
        bstages = [
            constp.tile([1, D], F32, tag=f"bstage{i}", name=f"bstage{i}")
            for i in range(4)
        ]
        for i in range(4):
            nc.gpsimd.dma_start(bstages[i][:], smalls[i : i + 1, :])

        for eng, wall, dram in (
            (nc.sync, wq_all, wq),
            (nc.sync, wk_all, wk),
            (nc.gpsimd, wv_all, wv),
            (nc.gpsimd, wo_all, wo),
        ):
            eng.dma_start(
                wall.rearrange("p (t d) -> p t d", d=D),
                dram.rearrange("(t p) d -> p t d", p=P),
            )

        def wt(wall, mt, c0, c1):
            return wall[:, mt * D + c0 : mt * D + c1]

        # ---- constants -----------------------------------------------------
        ones_stage = constp.tile([1, P], F32, tag="onesstage")
        nc.vector.memset(ones_stage[:], 1.0)
        ones_row = constp.tile([1, P], F32R, tag="onesrow")
        nc.vector.tensor_copy(ones_row[:], ones_stage[:])
        ones_col = constp.tile([P, H], F32, tag="onescol")
        nc.vector.memset(ones_col[:], 1.0)

        bq_row = constp.tile([1, D], F32R, tag="bqrow")
        bk_row = constp.tile([1, D], F32R, tag="bkrow")
        bv_row = constp.tile([1, D], F32R, tag="bvrow")
        bo_row = constp.tile([1, D], F32R, tag="borow")
        for row, stage in zip((bq_row, bk_row, bv_row, bo_row), bstages):
            nc.vector.tensor_copy(row[:], stage[:])

        # bq/bk as per-partition bias columns: transpose each 128-chunk of the
        # bias row via a tiny N=1 matmul (lhsT = row slice, rhs = ones[1,1])
        bq_t = constp.tile([P, MT], F32, tag="bq")  # col hdb = bias block
        bk_t = constp.tile([P, MT], F32, tag="bk")
        bcol_ps = psflow.tile([P, 512], F32, tag="ps", name="bcol_ps")
        for hdb in range(MT):
            nc.tensor.matmul(
                bcol_ps[:, 2 * hdb : 2 * hdb + 2],
                bq_row[:, hdb * P : (hdb + 1) * P],
                ones_row[:, 0:2],
                start=True, stop=True,
            )
            nc.tensor.matmul(
                bcol_ps[:, 2 * MT + 2 * hdb : 2 * MT + 2 * hdb + 2],
                bk_row[:, hdb * P : (hdb + 1) * P],
                ones_row[:, 0:2],
                start=True, stop=True,
            )
        for dst, c0 in ((bq_t, 0), (bk_t, 2 * MT)):
            src = bass.AP(
                bcol_ps.tensor, bcol_ps.offset + c0,
                [bcol_ps.ap[0], [2, MT]],
            )
            nc.vector.tensor_copy(dst[:], src)

        # broadcast bv/bo rows to all partitions via K=1 outer-product matmul
        bv_b = constp.tile([P, D], F32, tag="bvb")
        bo_b = constp.tile([P, D], F32, tag="bob")
        for row, bcast in ((bv_row, bv_b), (bo_row, bo_b)):
            for c0, c1 in ((0, 512), (512, 768)):
                bps = psflow.tile([P, 512], F32, tag="ps", name="bps")
                nc.tensor.matmul(
                    bps[:, : c1 - c0],
                    ones_row[:],
                    row[:, c0:c1],
                    start=True,
                    stop=True,
                )
                nc.vector.tensor_copy(bcast[:, c0:c1], bps[:, : c1 - c0])

        # ---- projections ---------------------------------------------------
        qts = [qtp.tile([P, S], MMDT, tag=f"qt{i}", name=f"qt{i}") for i in range(MT)]
        kts = [ktp.tile([P, S], MMDT, tag=f"kt{i}", name=f"kt{i}") for i in range(MT)]
        vps = [
            vpp.tile([P, H * 65], MMDT, tag=f"vp{st}", name=f"vp{st}")
            for st in range(NKT)
        ]

        def proj_qk_piece(wall, b_t, dst, sc, hdb):
            s0 = sc * 512
            ps0 = psflow.tile([P, 512], F32, tag="ps", name="pj0")
            for mt in range(MT):
                nc.tensor.matmul(
                    ps0[:], wt(wall, mt, hdb * P, (hdb + 1) * P),
                    xts[mt][:, s0 : s0 + 512],
                    start=(mt == 0), stop=(mt == MT - 1),
                )
            bsl = b_t[:, hdb : hdb + 1]
            bb = bass.AP(bsl.tensor, bsl.offset, [bsl.ap[0], [0, 512]])
            nc.vector.tensor_tensor(
                dst[hdb][:, s0 : s0 + 512], ps0[:], bb, op=add
            )

        def proj_qk_chunk(wall, b_t, dst, sc):
            for hdb in range(MT):
                proj_qk_piece(wall, b_t, dst, sc, hdb)

        def proj_v(st):
            vv = vps[st].rearrange("p (h c) -> p h c", c=65)
            nc.vector.tensor_copy(
                vv[:, :, 64:65],
                ones_col.rearrange("p (h c) -> p h c", c=1),
            )
            ps0 = psflow.tile([P, 512], F32, tag="ps", name="pv0")
            ps1 = psflow.tile([P, 512], F32, tag="ps", name="pv1")
            for mt in range(MT):
                lx = xts[mt][:, st * P : (st + 1) * P]
                nc.tensor.matmul(
                    ps0[:], lx, wt(wv_all, mt, 0, 512),
                    start=(mt == 0), stop=(mt == MT - 1),
                )
                nc.tensor.matmul(
                    ps1[:, 0:256], lx, wt(wv_all, mt, 512, 768),
                    start=(mt == 0), stop=(mt == MT - 1),
                )
            bsrc = bv_b.rearrange("p (h c) -> p h c", c=DH)
            nc.vector.tensor_tensor(
                vv[:, 0:8, 0:DH],
                ps0.rearrange("p (h c) -> p h c", c=DH),
                bsrc[:, 0:8, :],
                op=add,
            )
            nc.vector.tensor_tensor(
                vv[:, 8:12, 0:DH],
                ps1[:, 0:256].rearrange("p (h c) -> p h c", c=DH),
                bsrc[:, 8:12, :],
                op=add,
            )

        # ---- attention core -------------------------------------------------
        # per (pp, hp): loop k-tiles; causal handled by narrowing every op to
        # the live q-range [lo:QH] (lo = 128*kt - 512*pp clamped at 0) plus a
        # [128,128] triangular mask on diagonal tiles only.
        def attn_core(pp, hp, wstack):
            q0 = pp * QH
            nkt1 = 4 * pp + 4
            pvs = psacc.tile([65, 2 * QH], F32, tag="pv", name=f"pv{pp}_{hp}")
            for kt in range(nkt1):
                off = P * kt - QH * pp
                lo = max(0, off)
                w = QH - lo
                scp = scpp.tile([P, 2 * QH], F32, tag="scp", name="scp")
                # the pair's two matmuls sit on disjoint PE row groups and
                # disjoint PSUM banks of one 2-bank tile
                for sub in range(2):
                    r0 = sub * 64
                    nc.tensor.matmul(
                        scp[:, sub * QH + lo : (sub + 1) * QH],
                        kts[hp][r0 : r0 + 64, kt * P : (kt + 1) * P],
                        qts[hp][r0 : r0 + 64, q0 + lo : q0 + QH],
                        start=True,
                        stop=True,
                        tile_position=(r0, 0),
                    )
                est = expp.tile([P, 2 * QH], MMDT, tag="est", name="est")
                if lo == 0:
                    nc.scalar.activation(est[:], scp[:], Exp, scale=SCALE)
                else:
                    sin = bass.AP(
                        scp.tensor, scp.offset + lo,
                        [scp.ap[0], [QH, 2], [1, w]],
                    )
                    sout = bass.AP(
                        est.tensor, est.offset + lo,
                        [est.ap[0], [QH, 2], [1, w]],
                    )
                    nc.scalar.activation(sout, sin, Exp, scale=SCALE)
                if off >= 0:
                    # diagonal tile: triangular mask on cols [off:off+P]
                    for sub in range(2):
                        b0 = sub * QH + off
                        nc.gpsimd.affine_select(
                            est[:, b0 : b0 + P], est[:, b0 : b0 + P],
                            pattern=[[1, P]],
                            compare_op=is_ge, fill=0.0,
                            base=0,
                            channel_multiplier=-1,
                        )
                for sub in range(2):
                    h = 2 * hp + sub
                    nc.tensor.matmul(
                        pvs[:, sub * QH + lo : (sub + 1) * QH],
                        vps[kt][:, h * 65 : (h + 1) * 65],
                        est[:, sub * QH + lo : (sub + 1) * QH],
                        start=(kt == 0),
                        stop=(kt == nkt1 - 1),
                        skip_group_check=True,
                    )
            # immediate stash frees the PV banks: unnormalized rows into
            # wstack (bf16); reciprocal of the denominator row straight off
            # PSUM partition 64 (32-aligned base) into an f32r row
            for sub in range(2):
                nc.vector.tensor_copy(
                    wstack[hp][sub * 64 : (sub + 1) * 64, :],
                    pvs[0:64, sub * QH : (sub + 1) * QH],
                )
            # 1/d = exp(-ln d), Ln read straight off the PSUM denominator
            # row (Ln and Exp co-reside in one ACT table set: no reloads)
            lnr = smallp.tile([1, 2 * QH], F32, tag="lnr", bufs=3,
                              name=f"lnr{pp}_{hp}")
            nc.scalar.activation(
                lnr[:], pvs[64:65, :], mybir.ActivationFunctionType.Ln
            )
            srec = smallp.tile([1, 2 * QH], F32R, tag="srec", bufs=8,
                               name=f"srec{pp}_{hp}")
            nc.scalar.activation(srec[:], lnr[:], Exp, scale=-1.0)
            return srec

        def norm_apply(hp, srec, wstack):
            # broadcast each head's reciprocal row to 64 partitions via a
            # K=1 matmul (PSUM dst must start at partition 0), then scale
            for sub in range(2):
                rb = psflow.tile([P, 512], F32, tag="ps", name="rb")
                nc.tensor.matmul(
                    rb[0:64, :],
                    ones_row[:, 0:64],
                    srec[:, sub * QH : (sub + 1) * QH],
                    start=True, stop=True,
                )
                nc.vector.tensor_tensor(
                    wstack[hp][sub * 64 : (sub + 1) * 64, :],
                    wstack[hp][sub * 64 : (sub + 1) * 64, :],
                    rb[0:64, :], op=mult,
                )

        def outproj_sub(pp, wstack, sub):
            q0 = pp * QH
            opsa = psflow.tile([P, 512], F32, tag="ps", name="opa_t")
            opsb = psflow.tile([P, 512], F32, tag="ps", name="opb_t")
            for hdt in range(MT):
                lw = wstack[hdt][:, sub * P : (sub + 1) * P]
                nc.tensor.matmul(
                    opsa[:], lw, wt(wo_all, hdt, 0, 512),
                    start=(hdt == 0), stop=(hdt == MT - 1),
                )
                nc.tensor.matmul(
                    opsb[:, 0:256], lw, wt(wo_all, hdt, 512, 768),
                    start=(hdt == 0), stop=(hdt == MT - 1),
                )
            osb = outp.tile([P, D], F32, tag="osb")
            nc.vector.tensor_tensor(
                osb[:, 0:512], opsa[:], bo_b[:, 0:512], op=add
            )
            nc.vector.tensor_tensor(
                osb[:, 512:768], opsb[:, 0:256], bo_b[:, 512:768], op=add
            )
            nc.sync.dma_start(
                y[q0 + sub * P : q0 + (sub + 1) * P, :], osb[:]
            )

        # ---- emission order -------------------------------------------------
        proj_qk_chunk(wq_all, bq_t, qts, 0)
        proj_qk_chunk(wk_all, bk_t, kts, 0)
        for st in range(4):
            proj_v(st)

        wstack0 = [
            wsp.tile([P, QH], MMDT, tag="ws", name=f"ws0_{i}")
            for i in range(MT)
        ]
        wstack1 = [
            wsp.tile([P, QH], MMDT, tag="ws", name=f"ws1_{i}")
            for i in range(MT)
        ]
        # q-half 0 attention; interleave half-1 projections between head pairs
        srecs0 = []
        for hp in range(MT):
            srecs0.append(attn_core(0, hp, wstack0))
            proj_qk_piece(wq_all, bq_t, qts, 1, hp)
            proj_qk_piece(wk_all, bk_t, kts, 1, hp)
            if hp < 4:
                proj_v(4 + hp)

        # q-half 1 attention; interleave half-0 norm + out-projection
        srecs1 = []
        for hp in range(MT):
            srecs1.append(attn_core(1, hp, wstack1))
            if hp == 0:
                for hp0 in range(3):
                    norm_apply(hp0, srecs0[hp0], wstack0)
            elif hp == 1:
                for hp0 in range(3, MT):
                    norm_apply(hp0, srecs0[hp0], wstack0)
            else:
                outproj_sub(0, wstack0, hp - 2)
        for hp in range(MT):
            norm_apply(hp, srecs1[hp], wstack1)
        for sub in range(4):
            outproj_sub(1, wstack1, sub)
    return nc


_NC_CACHE = None
LAST_EXEC_NS = None


def _get_nc():
    global _NC_CACHE
    if _NC_CACHE is None:
        _NC_CACHE = build_nc()
    return _NC_CACHE


def kernel(
    normalized_resid_pre, W_Q, W_K, W_V, W_O, b_Q, b_K, b_V, b_O
) -> np.ndarray:
    global LAST_EXEC_NS
    bf = ml_dtypes.bfloat16
    x = np.asarray(normalized_resid_pre, np.float32)
    xT = np.ascontiguousarray(x.transpose(0, 2, 1)).astype(bf)  # [b, D, S]
    wq = np.asarray(W_Q, np.float32).transpose(1, 0, 2).reshape(D, D).astype(bf)
    wk = np.asarray(W_K, np.float32).transpose(1, 0, 2).reshape(D, D).astype(bf)
    wv = np.asarray(W_V, np.float32).transpose(1, 0, 2).reshape(D, D).astype(bf)
    wo = np.asarray(W_O, np.float32).reshape(D, D).astype(bf)
    smalls = np.zeros((4, D), np.float32)
    smalls[0] = np.asarray(b_Q, np.float32).reshape(D)
    smalls[1] = np.asarray(b_K, np.float32).reshape(D)
    smalls[2] = np.asarray(b_V, np.float32).reshape(D)
    smalls[3] = np.asarray(b_O, np.float32).reshape(D)

    nc = _get_nc()
    in_maps = [
        {
            "xT": xT[i],
            "wq": wq, "wk": wk, "wv": wv, "wo": wo,
            "smalls": smalls,
        }
        for i in range(N_CORES)
    ]
    trace = os.environ.get("KERNEL_TRACE", "0") == "1"
    res = run_bass_kernel_spmd(
        nc, in_maps, list(range(N_CORES)), trace=trace
    )
    LAST_EXEC_NS = res.exec_time_ns
    out = np.stack(
        [res.results[i]["y"].astype(np.float32) for i in range(N_CORES)], axis=0
    )
    return out
